# revision 1
# baseline (speedup 1.0000x reference)
"""Trainium (trn2) kernel for CurvedRoIExtractor (nn_CurvedRoIExtractor_28295244546862).

kernel(**inputs) takes the FULL inputs (as produced by setup_inputs()) and
returns the FULL output [2, 256, 256, 3, 16] f32.

Sharding: 8 cores = (batch b in {0,1}) x (64-roi quarter).  Features for the
core's batch are passed pre-transposed (channel-last, levels concatenated) so
the device can fetch the 4 bilinear-neighbor pixel rows of every sample point
with nc.gpsimd.dma_gather (1 KB contiguous per gathered pixel).  The weighted
sum over (level, neighbor) runs on TensorE via masked block-diagonal
matmuls accumulated in PSUM.  See the builder docstring below for layout
details.
"""

from contextlib import ExitStack

import numpy as np

import concourse.bass as bass
import concourse.mybir as mybir
import concourse.tile as tile
from concourse import library_config
from concourse.bass_utils import run_bass_kernel_spmd
from concourse.tile import add_dep_helper

F32 = mybir.dt.float32
F32R = mybir.dt.float32r
I16 = mybir.dt.int16
AOP = mybir.AluOpType

# (W, H, base row) of each feature level inside the concatenated table
LEVELS = [
    (160, 160, 0),
    (80, 80, 25600),
    (40, 40, 32000),
    (20, 20, 33600),
]
ROWS = 34048          # 34000 + padding rows (only weight-0 neighbors land there)
C = 256               # channels
BS = 2
NROI_TOTAL = 256
WP = 16
OUT_H = 3
NPTS = 3072           # per core: 64 rois * 3 * 16
NG_CHUNK = 8          # 32-point groups per chunk (1024-idx gathers: single_packet fast path)
MM_DTYPE = F32


def _fix_waits(nc, max_waits=1):
    """The walrus build in this env rejects >1 sem wait per instruction;
    spill extras onto preceding NOPs on the same engine."""
    for func in nc.m.functions:
        for bb in func.blocks:
            insts = bb.instructions
            for ins in list(insts):
                si = ins.sync_info
                if si is None:
                    continue
                w = list(si.on_wait)
                if len(w) > max_waits:
                    si.on_wait = w[:max_waits]
                    pos = insts.index(ins)
                    extra = w[max_waits:]
                    for k in range(0, len(extra), max_waits):
                        nop = mybir.InstNoOp(
                            name=f"{ins.name}-wf{k}",
                            engine=ins.engine,
                            bass_nofuse=True,
                            sync_info=mybir.SyncInfo(
                                on_wait=extra[k : k + max_waits], on_update=[]
                            ),
                        )
                        insts.insert(pos, nop)
                        pos += 1


def _build_kernel(levels=None, rows=ROWS, npts=NPTS, ng_chunk=NG_CHUNK,
                  mm_dtype=MM_DTYPE, fix=True):
    """Per-core program.

    Point order p = h*1024 + w*64 + roi' ; j = p%32, group g = p//32.
    Gather token order per level: t = 128*g + 32*n + j (n = bilinear
    neighbor 00,01,10,11) -> token t lands at partition t%128, block t//128,
    so group g's 128 (neighbor, point) rows fill all 128 partitions of
    block g.  Weighted sum over (level, neighbor): per (group, level) a
    matmul with masked block-diagonal lhsT[q, j'] = (q%32==j') *
    w_{l,q//32}[32g + q%32], accumulated over levels into PSUM [32, 256].
    """
    if levels is None:
        levels = LEVELS
    nlvl = len(levels)
    ngrp = npts // 32
    assert npts % 32 == 0 and ngrp % ng_chunk == 0
    nchunk = ngrp // ng_chunk
    icols = ngrp * 8          # idx table cols (= 4*npts/16)
    ccols = ng_chunk * 8      # idx cols per chunk

    nc = bass.Bass("TRN2", target_bir_lowering=False, num_devices=8,
                   num_swdge_queues=4)
    tf = nc.dram_tensor("tfeats", [rows, C], F32, kind="ExternalInput")
    gxd = nc.dram_tensor("gx", [32, ngrp], F32, kind="ExternalInput")
    gyd = nc.dram_tensor("gy", [32, ngrp], F32, kind="ExternalInput")
    gx16d = nc.dram_tensor("gx16", [16, 2 * ngrp], F32, kind="ExternalInput")
    gy16d = nc.dram_tensor("gy16", [16, 2 * ngrp], F32, kind="ExternalInput")
    maskd = nc.dram_tensor("mask", [128, 32], F32, kind="ExternalInput")
    outd = nc.dram_tensor("out", [npts, C], F32, kind="ExternalOutput")

    with tile.TileContext(nc) as tc, ExitStack() as ctx:
        prep = ctx.enter_context(tc.tile_pool(name="prep", bufs=1))
        gpool = ctx.enter_context(tc.tile_pool(name="g", bufs=4))
        lpool = ctx.enter_context(tc.tile_pool(name="lhs", bufs=2))
        spool = ctx.enter_context(tc.tile_pool(name="stage", bufs=3))
        ppool = ctx.enter_context(tc.tile_pool(name="ps", bufs=8, space="PSUM"))

        nc.gpsimd.load_library(library_config.attnmlp)

        gx = prep.tile([32, ngrp], F32, tag="gx")
        gy = prep.tile([32, ngrp], F32, tag="gy")
        mask = prep.tile([128, 32], F32, tag="mask")
        gx16 = prep.tile([16, 2 * ngrp], F32, tag="gx16")
        gy16 = prep.tile([16, 2 * ngrp], F32, tag="gy16")
        nc.sync.dma_start(gx[:], gxd[:])
        nc.sync.dma_start(gy[:], gyd[:])
        nc.sync.dma_start(gx16[:], gx16d[:])
        nc.sync.dma_start(gy16[:], gy16d[:])
        nc.sync.dma_start(mask[:], maskd[:])

        wcol = []   # per level [128, ngrp] weights (partition q = 32n+j)
        idxr = []   # per level [128, icols] int16 idx tables (replicated)
        for l, (W, H, base) in enumerate(levels):
            x = prep.tile([32, ngrp], F32, tag="x")
            y = prep.tile([32, ngrp], F32, tag="y")
            # match the reference's rounding: ((g + 1) * 0.5) * (W - 1)
            nc.vector.tensor_scalar(x[:], gx[:], 1.0, 0.5, AOP.add, AOP.mult)
            nc.vector.tensor_scalar(x[:], x[:], float(W - 1), None, AOP.mult)
            nc.vector.tensor_scalar(y[:], gy[:], 1.0, 0.5, AOP.add, AOP.mult)
            nc.vector.tensor_scalar(y[:], y[:], float(H - 1), None, AOP.mult)
            # floor(v) = round(v) - (round(v) > v), round via +/- 2^23
            # (exact: v in [0, 2^15), fp32 RN)
            wx = prep.tile([32, ngrp], F32, tag="wx")
            wy = prep.tile([32, ngrp], F32, tag="wy")
            x0 = prep.tile([32, ngrp], F32, tag="x0")
            y0 = prep.tile([32, ngrp], F32, tag="y0")
            M23 = 8388608.0
            for v, v0, frac in ((x, x0, wx), (y, y0, wy)):
                nc.vector.tensor_scalar(v0[:], v[:], M23, -M23, AOP.add, AOP.add)
                nc.vector.tensor_tensor(frac[:], v0[:], v[:], AOP.is_gt)
                nc.vector.tensor_tensor(v0[:], v0[:], frac[:], AOP.subtract)
                nc.vector.tensor_tensor(frac[:], v[:], v0[:], AOP.subtract)
            # idx tables, built on the 16-partition point layout
            # ([r, 2g+u] = point 32g+16u+r) so the wrapped table needs only
            # DVE strided writes, no element-grain DMA.
            x_ = prep.tile([16, 2 * ngrp], F32, tag="x_")
            y_ = prep.tile([16, 2 * ngrp], F32, tag="y_")
            nc.vector.tensor_scalar(x_[:], gx16[:], 1.0, 0.5, AOP.add, AOP.mult)
            nc.vector.tensor_scalar(x_[:], x_[:], float(W - 1), None, AOP.mult)
            nc.vector.tensor_scalar(y_[:], gy16[:], 1.0, 0.5, AOP.add, AOP.mult)
            nc.vector.tensor_scalar(y_[:], y_[:], float(H - 1), None, AOP.mult)
            t_ = prep.tile([16, 2 * ngrp], F32, tag="t_")
            i00 = prep.tile([16, 2 * ngrp], F32, tag="i00")
            for v, v0 in ((x_, t_), (y_, i00)):
                nc.vector.tensor_scalar(v0[:], v[:], M23, -M23, AOP.add, AOP.add)
                nc.vector.tensor_tensor(v[:], v0[:], v[:], AOP.is_gt)
                nc.vector.tensor_tensor(v0[:], v0[:], v[:], AOP.subtract)
            # i00 = y0*W + x0 (t_ = x0, i00 = y0)
            nc.vector.tensor_scalar(i00[:], i00[:], float(W), None, AOP.mult)
            nc.vector.tensor_tensor(i00[:], i00[:], t_[:], AOP.add)

            # wrapped int16 idx table [16, icols]: [r, 8g+2n+u] =
            #   i00[r, 2g+u] + off_n
            i16 = prep.tile([16, icols], I16, tag=f"i16_{l}")
            dview = i16[:].rearrange("p (g n u) -> p n g u", n=4, u=2)
            sview = i00[:].rearrange("p (g u) -> p g u", u=2)
            for n, off in enumerate([0.0, 1.0, float(W), float(W + 1)]):
                nc.vector.tensor_scalar(dview[:, n], sview, off, None, AOP.add)
            rep = prep.tile([128, icols], I16, tag=f"irep{l}")
            for k in range(8):
                nc.sync.dma_start(rep[16 * k : 16 * k + 16, :], i16[:])
            idxr.append(rep)
            mx = prep.tile([32, ngrp], F32, tag="mx")  # 1-wx
            my = prep.tile([32, ngrp], F32, tag="my")  # 1-wy
            nc.vector.tensor_scalar(mx[:], wx[:], -1.0, 1.0, AOP.mult, AOP.add)
            nc.vector.tensor_scalar(my[:], wy[:], -1.0, 1.0, AOP.mult, AOP.add)

            wc = prep.tile([128, ngrp], F32, tag=f"wcol{l}")
            nc.vector.tensor_tensor(wc[0:32, :], mx[:], my[:], AOP.mult)
            nc.vector.tensor_tensor(wc[32:64, :], wx[:], my[:], AOP.mult)
            nc.vector.tensor_tensor(wc[64:96, :], mx[:], wy[:], AOP.mult)
            nc.vector.tensor_tensor(wc[96:128, :], wx[:], wy[:], AOP.mult)
            wcol.append(wc)



        chunks = []
        g0 = 0
        while g0 < ngrp:
            cs = min(ng_chunk, ngrp - g0)
            chunks.append((g0, cs))
            g0 += cs
        for ch, (gc0, cs) in enumerate(chunks):
            gts = []
            for l, (W, H, base) in enumerate(levels):
                gt = gpool.tile([128, ng_chunk, C], F32, tag=f"g{l}")
                hi = min(base + W * H + 2 * W + 2, rows)
                nc.gpsimd.dma_gather(
                    out_ap=gt[:, 0:cs, :],
                    in_ap=tf[base:hi, :],
                    idxs_ap=idxr[l][:, gc0 * 8 : (gc0 + cs) * 8],
                    num_idxs=cs * 128,
                    num_idxs_reg=cs * 128,
                    elem_size=C,
                    queue_num=(ch * nlvl + l) % 4,
                )
                gts.append(gt)
            lhs = []
            for l in range(nlvl):
                lt = lpool.tile([128, ng_chunk * 32], F32, tag=f"w{l}")
                wslice = wcol[l][:, gc0 : gc0 + cs]
                nc.vector.tensor_tensor(
                    lt[:].rearrange("p (g k) -> p g k", k=32)[:, 0:cs, :],
                    mask[:].unsqueeze(1).to_broadcast([128, cs, 32]),
                    wslice.to_broadcast([128, cs, 32]),
                    AOP.mult,
                )
                lhs.append(lt)

            prev_mm = None
            for cl in range(ng_chunk // 4):
                ps = ppool.tile([128, C], F32, tag="ps")
                for a in range(4):
                    gi = cl * 4 + a
                    for l in range(nlvl):
                        mm = nc.tensor.matmul(
                            ps[32 * a : 32 * a + 32, :],
                            lhs[l][:, 32 * gi : 32 * (gi + 1)].bitcast(mm_dtype),
                            gts[l][:, gi, :].bitcast(mm_dtype),
                            start=(l == 0),
                            stop=(l == nlvl - 1),
                            tile_position=(0, 32 * a),
                        )
                        # Force PE order: accumulation chains sharing a PSUM
                        # bank must not interleave (start=True clears the
                        # whole bank's has_written bits).
                        if prev_mm is not None:
                            add_dep_helper(mm.ins, prev_mm.ins, sync=False)
                        prev_mm = mm
                st = spool.tile([128, C], F32, tag="st")
                nc.vector.tensor_copy(out=st[:], in_=ps[:])
                row0 = ch * ng_chunk * 32 + cl * 128
                nc.sync.dma_start(outd[row0 : row0 + 128, :], st[:])

    mybir.codegen_inst_isa_subclasses(nc)
    if fix:
        _fix_waits(nc)
    return nc


# ---------------------------------------------------------------------------
# Host-side prep

def _host_prep_points(center_b, boundary_b, roi0, nroi):
    bp = boundary_b[roi0 : roi0 + nroi]      # [nroi, Wp, 4]
    cp = center_b[roi0 : roi0 + nroi]        # [nroi, Wp, 2]
    sp = np.stack([bp[..., 0:2], cp, bp[..., 2:4]], axis=1)  # [nroi,3,Wp,2]
    gxa = np.ascontiguousarray(sp[..., 0].transpose(1, 2, 0)).reshape(-1)
    gya = np.ascontiguousarray(sp[..., 1].transpose(1, 2, 0)).reshape(-1)
    npts = gxa.size
    gx = gxa.reshape(npts // 32, 32).T.copy()
    gy = gya.reshape(npts // 32, 32).T.copy()
    g = npts // 32
    gx16 = gxa.reshape(g, 2, 16).transpose(2, 0, 1).reshape(16, 2 * g)
    gy16 = gya.reshape(g, 2, 16).transpose(2, 0, 1).reshape(16, 2 * g)
    return (gx.astype(np.float32), gy.astype(np.float32),
            np.ascontiguousarray(gx16, np.float32),
            np.ascontiguousarray(gy16, np.float32))


def _host_mask():
    q = np.arange(128)[:, None] % 32
    j = np.arange(32)[None, :]
    return (q == j).astype(np.float32)


def _host_tfeats(feats_b_list, rows=ROWS):
    parts = [np.ascontiguousarray(f.reshape(f.shape[0], -1).T)
             for f in feats_b_list]
    tfx = np.concatenate(parts, axis=0)
    pad = rows - tfx.shape[0]
    if pad:
        tfx = np.concatenate(
            [tfx, np.zeros((pad, tfx.shape[1]), np.float32)], axis=0)
    return np.ascontiguousarray(tfx.astype(np.float32))


_CACHE = {}


def _get_nc():
    if "nc" not in _CACHE:
        _CACHE["nc"] = _build_kernel()
    return _CACHE["nc"]


def kernel(feats0, feats1, feats2, feats3, center_points, boundary_points,
           _want_trace=False, _trace_dir=None):
    feats0 = np.asarray(feats0, dtype=np.float32)
    feats1 = np.asarray(feats1, dtype=np.float32)
    feats2 = np.asarray(feats2, dtype=np.float32)
    feats3 = np.asarray(feats3, dtype=np.float32)
    center_points = np.asarray(center_points, dtype=np.float32)
    boundary_points = np.asarray(boundary_points, dtype=np.float32)

    nc = _get_nc()
    mask = _host_mask()
    tfeats = [
        _host_tfeats([feats0[b], feats1[b], feats2[b], feats3[b]])
        for b in range(BS)
    ]
    nroi = NROI_TOTAL // 4  # 64 rois per core
    in_maps = []
    for core in range(8):
        b = core // 4
        roi0 = (core % 4) * nroi
        gx, gy, gx16, gy16 = _host_prep_points(
            center_points[b], boundary_points[b], roi0, nroi)
        in_maps.append(
            {"tfeats": tfeats[b], "gx": gx, "gy": gy,
             "gx16": gx16, "gy16": gy16, "mask": mask})

    kwargs = {}
    if _want_trace:
        kwargs = {"trace": True}
        if _trace_dir is not None:
            kwargs["tmpdir"] = _trace_dir
    res = run_bass_kernel_spmd(nc, in_maps, core_ids=list(range(8)), **kwargs)

    out = np.empty((BS, NROI_TOTAL, C, OUT_H, WP), np.float32)
    for core in range(8):
        b = core // 4
        roi0 = (core % 4) * nroi
        dev = res.results[core]["out"]          # [NPTS, C], rows (h, w, roi')
        o = dev.reshape(OUT_H, WP, nroi, C).transpose(2, 3, 0, 1)
        out[b, roi0 : roi0 + nroi] = o
    if _want_trace:
        return out, res
    return out



# revision 3
# speedup vs baseline: 1.4822x; 1.4822x over previous
"""Trainium (trn2) kernel for CurvedRoIExtractor (nn_CurvedRoIExtractor_28295244546862).

kernel(**inputs) takes the FULL inputs (as produced by setup_inputs()) and
returns the FULL output [2, 256, 256, 3, 16] f32.

Sharding: 8 cores = (batch b in {0,1}) x (64-roi quarter).  The core's
feature maps are pre-transposed on the host to a channel-last fp16 table
[34000, 256] (levels concatenated).  For every sample point the device
fetches the two ADJACENT-pixel pairs (x0,x1)@y0 and (x0,x1)@y1 per level
with nc.gpsimd.dma_gather — one 1 KB descriptor per pair (elem_size=512
fp16 elems, elem_step=256 so the pair window overlaps the row grid).
Bilinear weights (host-precomputed, fp16) are applied on DVE as a
per-(pair,side) broadcast multiply, then the 16 weighted slices per point
(4 levels x pair x side) are accumulated into PSUM by identity-stationary
fp16 matmuls.  PSUM (f32) is staged to fp16 on the Scalar engine and
DMA'd out; the host upcasts.
"""

from contextlib import ExitStack

import numpy as np

import concourse.bass as bass
import concourse.mybir as mybir
import concourse.tile as tile
from concourse import library_config
from concourse.bass_utils import run_bass_kernel_spmd
from concourse.tile import add_dep_helper

F32 = mybir.dt.float32
F16 = mybir.dt.float16
I16 = mybir.dt.int16
AOP = mybir.AluOpType

# (W, H, base row) of each feature level inside the concatenated table
LEVELS = [
    (160, 160, 0),
    (80, 80, 25600),
    (40, 40, 32000),
    (20, 20, 33600),
]
ROWS = 34000
C = 256               # channels
BS = 2
NROI_TOTAL = 256
WP = 16
OUT_H = 3
NPTS = 3072           # per core: 64 rois * 3 * 16
NG = 4                # 128-point groups per chunk (-> 1024-idx gathers)
NCHUNK = NPTS // (NG * 128)   # 6
NLVL = len(LEVELS)
NSEG = NCHUNK * NLVL  # gather segments


def _fix_waits(nc, max_waits=1):
    """The walrus build in this env rejects >1 sem wait per instruction;
    spill extras onto preceding NOPs on the same engine."""
    for func in nc.m.functions:
        for bb in func.blocks:
            insts = bb.instructions
            for ins in list(insts):
                si = ins.sync_info
                if si is None:
                    continue
                w = list(si.on_wait)
                if len(w) > max_waits:
                    si.on_wait = w[:max_waits]
                    pos = insts.index(ins)
                    extra = w[max_waits:]
                    for k in range(0, len(extra), max_waits):
                        nop = mybir.InstNoOp(
                            name=f"{ins.name}-wf{k}",
                            engine=ins.engine,
                            bass_nofuse=True,
                            sync_info=mybir.SyncInfo(
                                on_wait=extra[k : k + max_waits], on_update=[]
                            ),
                        )
                        insts.insert(pos, nop)
                        pos += 1


def _build_kernel(fix=True):
    """Per-core program.

    Point order p = h*1024 + w*64 + roi'.  Chunk ch covers points
    [ch*512, ch*512+512) as NG=4 groups of 128.  Gather token order per
    (chunk, level): t = (g*2 + tb)*128 + j  (tb: 0=top pair @y0, 1=bottom
    pair @y1), landing at partition j, block g*2+tb of a [128, 8, 512]
    fp16 tile.  DVE multiplies each 256-ch half of every block by its
    host-computed bilinear weight (broadcast [128, 16] -> [128, 16, 256]).
    Identity-stationary matmuls then accumulate the 16 slices per
    (group, level ...) into PSUM [128, 256].
    """
    nc = bass.Bass("TRN2", target_bir_lowering=False, num_devices=8,
                   num_swdge_queues=4)
    tf = nc.dram_tensor("tfeats", [ROWS, C], F16, kind="ExternalInput")
    idxd = nc.dram_tensor("idx", [128, NSEG * 64], I16, kind="ExternalInput")
    wtd = nc.dram_tensor("wt", [128, NSEG * 16], F16, kind="ExternalInput")
    identd = nc.dram_tensor("ident", [128, 128], F16, kind="ExternalInput")
    outd = nc.dram_tensor("out", [NPTS, C], F16, kind="ExternalOutput")
    tf_h = tf[:].tensor

    with tile.TileContext(nc) as tc, ExitStack() as ctx:
        prep = ctx.enter_context(tc.tile_pool(name="prep", bufs=1))
        gpool = ctx.enter_context(tc.tile_pool(name="g", bufs=2))
        spool = ctx.enter_context(tc.tile_pool(name="stage", bufs=4))
        ppool = ctx.enter_context(tc.tile_pool(name="ps", bufs=6, space="PSUM"))

        nc.gpsimd.load_library(library_config.attnmlp)

        idxt = prep.tile([128, NSEG * 64], I16, tag="idx")
        wtt = prep.tile([128, NSEG * 16], F16, tag="wt")
        ident = prep.tile([128, 128], F16, tag="ident")
        nc.sync.dma_start(idxt[:], idxd[:])
        nc.sync.dma_start(wtt[:], wtd[:])
        nc.sync.dma_start(ident[:], identd[:])

        prev_mm = None
        for ch in range(NCHUNK):
            gts = []
            for l, (W, H, base) in enumerate(LEVELS):
                gt = gpool.tile([128, NG * 2, 2 * C], F16, tag=f"g{l}")
                seg = ch * NLVL + l
                # overlapping pair window: row stride C, window 2*C
                in_ap = bass.AP(tf_h, base * C, [[C, W * H - 1], [1, 2 * C]])
                nc.gpsimd.dma_gather(
                    out_ap=gt[:],
                    in_ap=in_ap,
                    idxs_ap=idxt[:, seg * 64 : (seg + 1) * 64],
                    num_idxs=NG * 256,
                    num_idxs_reg=NG * 256,
                    elem_size=2 * C,
                    elem_step=C,
                    queue_num=seg % 4,
                )
                # bilinear weights: per (block, side) broadcast over channels
                gv = gt[:].rearrange("p b (s c) -> p (b s) c", s=2)
                wv = wtt[:, seg * 16 : (seg + 1) * 16]
                nc.vector.tensor_tensor(
                    gv,
                    gv,
                    wv.unsqueeze(2).to_broadcast([128, NG * 4, C]),
                    AOP.mult,
                )
                gts.append(gt)
            for g in range(NG):
                ps = ppool.tile([128, C], F32, tag="ps")
                k = 0
                for l in range(NLVL):
                    for tb in range(2):
                        for s in range(2):
                            mm = nc.tensor.matmul(
                                ps[:],
                                ident[:],
                                gts[l][:, 2 * g + tb, s * C : (s + 1) * C],
                                start=(k == 0),
                                stop=(k == 4 * NLVL - 1),
                            )
                            # PSUM-bank accumulation chains must not
                            # interleave (start=True clears the whole
                            # bank's has_written bits) -> force PE order.
                            if prev_mm is not None:
                                add_dep_helper(mm.ins, prev_mm.ins, sync=False)
                            prev_mm = mm
                            k += 1
                so = spool.tile([128, C], F16, tag="st")
                nc.scalar.activation(so[:], ps[:],
                                     mybir.ActivationFunctionType.Copy)
                row0 = (ch * NG + g) * 128
                nc.sync.dma_start(outd[row0 : row0 + 128, :], so[:])

    mybir.codegen_inst_isa_subclasses(nc)
    if fix:
        _fix_waits(nc)
    return nc


# ---------------------------------------------------------------------------
# Host-side prep

def _host_prep_points(center_b, boundary_b, roi0, nroi):
    """Returns (idx [128, NSEG*64] i16, wt [128, NSEG*16] f16) for one core."""
    bp = boundary_b[roi0 : roi0 + nroi]      # [nroi, Wp, 4]
    cp = center_b[roi0 : roi0 + nroi]        # [nroi, Wp, 2]
    sp = np.stack([bp[..., 0:2], cp, bp[..., 2:4]], axis=1)  # [nroi,3,Wp,2]
    gx = np.ascontiguousarray(sp[..., 0].transpose(1, 2, 0)).reshape(-1)
    gy = np.ascontiguousarray(sp[..., 1].transpose(1, 2, 0)).reshape(-1)
    gx = gx.astype(np.float32)
    gy = gy.astype(np.float32)

    idx = np.zeros((128, NSEG * 64), np.int16)
    wt = np.zeros((128, NSEG * 16), np.float16)
    for l, (W, H, base) in enumerate(LEVELS):
        # match the reference's fp32 rounding: ((g + 1) * 0.5) * (W - 1)
        x = ((gx + np.float32(1.0)) * np.float32(0.5)) * np.float32(W - 1)
        y = ((gy + np.float32(1.0)) * np.float32(0.5)) * np.float32(H - 1)
        x0 = np.floor(x)
        y0 = np.floor(y)
        wx = x - x0
        wy = y - y0
        it = (y0 * W + x0).astype(np.int32)          # top-pair row index
        ib = it + W                                  # bottom-pair row index
        w00 = (1 - wx) * (1 - wy)
        w10 = wx * (1 - wy)
        w01 = (1 - wx) * wy
        w11 = wx * wy
        for ch in range(NCHUNK):
            seg = ch * NLVL + l
            pts = np.arange(ch * NG * 128, (ch + 1) * NG * 128,
                            dtype=np.int64).reshape(NG, 128)
            tok = np.empty((NG * 2, 128), np.int32)
            tok[0::2] = it[pts]
            tok[1::2] = ib[pts]
            flat = tok.reshape(-1)
            wrapped = flat.reshape(-1, 16).T.astype(np.int16)   # [16, 64]
            idx[:, seg * 64 : (seg + 1) * 64] = np.tile(wrapped, (8, 1))
            wseg = np.empty((128, NG * 2, 2), np.float32)
            wseg[:, 0::2, 0] = w00[pts].T
            wseg[:, 0::2, 1] = w10[pts].T
            wseg[:, 1::2, 0] = w01[pts].T
            wseg[:, 1::2, 1] = w11[pts].T
            wt[:, seg * 16 : (seg + 1) * 16] = \
                wseg.reshape(128, NG * 4).astype(np.float16)
    return idx, wt


def _host_tfeats(feats_b_list):
    parts = [np.ascontiguousarray(f.reshape(f.shape[0], -1).T)
             for f in feats_b_list]
    tfx = np.concatenate(parts, axis=0)
    assert tfx.shape[0] == ROWS
    return np.ascontiguousarray(tfx.astype(np.float16))


_CACHE = {}


def _get_nc():
    if "nc" not in _CACHE:
        _CACHE["nc"] = _build_kernel()
    return _CACHE["nc"]


def kernel(feats0, feats1, feats2, feats3, center_points, boundary_points,
           _want_trace=False, _trace_dir=None):
    feats0 = np.asarray(feats0, dtype=np.float32)
    feats1 = np.asarray(feats1, dtype=np.float32)
    feats2 = np.asarray(feats2, dtype=np.float32)
    feats3 = np.asarray(feats3, dtype=np.float32)
    center_points = np.asarray(center_points, dtype=np.float32)
    boundary_points = np.asarray(boundary_points, dtype=np.float32)

    nc = _get_nc()
    ident = np.eye(128, dtype=np.float16)
    tfeats = [
        _host_tfeats([feats0[b], feats1[b], feats2[b], feats3[b]])
        for b in range(BS)
    ]
    nroi = NROI_TOTAL // 4  # 64 rois per core
    in_maps = []
    for core in range(8):
        b = core // 4
        roi0 = (core % 4) * nroi
        idx, wt = _host_prep_points(
            center_points[b], boundary_points[b], roi0, nroi)
        in_maps.append(
            {"tfeats": tfeats[b], "idx": idx, "wt": wt, "ident": ident})

    kwargs = {}
    if _want_trace:
        kwargs = {"trace": True}
        if _trace_dir is not None:
            kwargs["tmpdir"] = _trace_dir
    res = run_bass_kernel_spmd(nc, in_maps, core_ids=list(range(8)), **kwargs)

    out = np.empty((BS, NROI_TOTAL, C, OUT_H, WP), np.float32)
    for core in range(8):
        b = core // 4
        roi0 = (core % 4) * nroi
        dev = res.results[core]["out"]          # [NPTS, C] f16, rows (h, w, roi')
        o = dev.astype(np.float32).reshape(OUT_H, WP, nroi, C)
        out[b, roi0 : roi0 + nroi] = o.transpose(2, 3, 0, 1)
    if _want_trace:
        return out, res
    return out


# revision 4
# speedup vs baseline: 1.9151x; 1.2921x over previous
"""Trainium (trn2) kernel for CurvedRoIExtractor (nn_CurvedRoIExtractor_28295244546862).

kernel(**inputs) takes the FULL inputs (as produced by setup_inputs()) and
returns the FULL output [2, 256, 256, 3, 16] f32.

Sharding: 8 cores = (batch b in {0,1}) x (64-roi quarter).  The core's
feature maps are pre-transposed on the host to a channel-last fp16 table
[34000, 256] (levels concatenated).  For every sample point the device
fetches the two ADJACENT-pixel pairs (x0,x1)@y0 and (x0,x1)@y1 per level
with nc.gpsimd.dma_gather — one 1 KB descriptor per pair (elem_size=512
fp16 elems, elem_step=256 so the pair windows overlap on the row grid).
Gather token order t = g64*128 + tb*64 + j puts a 64-point group's top
pairs in partitions 0-63 and bottom pairs in 64-127; the bilinear
weighted sum then runs on TensorE as matmuls with a two-band masked
lhsT[q, j] = (q%64==j) * w_{tb(q), side}[j] (host-precomputed weights,
lhsT built on DVE as mask x broadcast), accumulating the 8 (level, side)
slices per group into PSUM.  PSUM (f32) is staged to fp16 on the Scalar
engine and DMA'd out; the host upcasts.
"""

from contextlib import ExitStack

import numpy as np

import concourse.bass as bass
import concourse.mybir as mybir
import concourse.tile as tile
from concourse import library_config
from concourse.bass_utils import run_bass_kernel_spmd
from concourse.tile import add_dep_helper

F32 = mybir.dt.float32
F16 = mybir.dt.float16
I16 = mybir.dt.int16
AOP = mybir.AluOpType

# (W, H, base row) of each feature level inside the concatenated table
LEVELS = [
    (160, 160, 0),
    (80, 80, 25600),
    (40, 40, 32000),
    (20, 20, 33600),
]
ROWS = 34000
C = 256               # channels
BS = 2
NROI_TOTAL = 256
WP = 16
OUT_H = 3
NPTS = 3072           # per core: 64 rois * 3 * 16
NG = 4                # 128-token groups per gather (-> 1024-idx gathers)
NCHUNK = NPTS // (NG * 128)   # 6 chunks of 512 points
NLVL = len(LEVELS)
NSEG = NCHUNK * NLVL  # gather segments


def _fix_waits(nc, max_waits=1):
    """The walrus build in this env rejects >1 sem wait per instruction;
    spill extras onto preceding NOPs on the same engine."""
    for func in nc.m.functions:
        for bb in func.blocks:
            insts = bb.instructions
            for ins in list(insts):
                si = ins.sync_info
                if si is None:
                    continue
                w = list(si.on_wait)
                if len(w) > max_waits:
                    si.on_wait = w[:max_waits]
                    pos = insts.index(ins)
                    extra = w[max_waits:]
                    for k in range(0, len(extra), max_waits):
                        nop = mybir.InstNoOp(
                            name=f"{ins.name}-wf{k}",
                            engine=ins.engine,
                            bass_nofuse=True,
                            sync_info=mybir.SyncInfo(
                                on_wait=extra[k : k + max_waits], on_update=[]
                            ),
                        )
                        insts.insert(pos, nop)
                        pos += 1


def _build_kernel(fix=True):
    """Per-core program.  See module docstring for the layout."""
    nc = bass.Bass("TRN2", target_bir_lowering=False, num_devices=8,
                   num_swdge_queues=4)
    tf = nc.dram_tensor("tfeats", [ROWS, C], F16, kind="ExternalInput")
    idxd = nc.dram_tensor("idx", [128, NSEG * 64], I16, kind="ExternalInput")
    wtd = nc.dram_tensor("wt", [128, NCHUNK * 64], F16, kind="ExternalInput")
    maskd = nc.dram_tensor("mask", [128, 64], F16, kind="ExternalInput")
    outd = nc.dram_tensor("out", [NPTS, C], F16, kind="ExternalOutput")
    tf_h = tf[:].tensor

    with tile.TileContext(nc) as tc, ExitStack() as ctx:
        prep = ctx.enter_context(tc.tile_pool(name="prep", bufs=1))
        gpool = ctx.enter_context(tc.tile_pool(name="g", bufs=3))
        spool = ctx.enter_context(tc.tile_pool(name="stage", bufs=4))
        ppool = ctx.enter_context(tc.tile_pool(name="ps", bufs=6, space="PSUM"))

        nc.gpsimd.load_library(library_config.attnmlp)

        idxt = prep.tile([128, NSEG * 64], I16, tag="idx")
        wtt = prep.tile([128, NCHUNK * 64], F16, tag="wt")
        mask = prep.tile([128, 64], F16, tag="mask")
        nc.sync.dma_start(idxt[:], idxd[:])
        nc.sync.dma_start(wtt[:], wtd[:])
        nc.sync.dma_start(mask[:], maskd[:])

        # lhsT tiles for every chunk, built upfront on DVE:
        # lt[q, l*16 + g64*2 + s, j] = mask[q, j] * w[q, chunk-col]
        lts = []
        for ch in range(NCHUNK):
            lt = prep.tile([128, 4 * 16, 64], F16, tag=f"lt{ch}")
            nc.vector.tensor_tensor(
                lt[:],
                mask[:].unsqueeze(1).to_broadcast([128, 64, 64]),
                wtt[:, ch * 64 : (ch + 1) * 64].unsqueeze(2)
                    .to_broadcast([128, 64, 64]),
                AOP.mult,
            )
            lts.append(lt)

        prev_mm = None
        for ch in range(NCHUNK):
            gts = []
            for l, (W, H, base) in enumerate(LEVELS):
                gt = gpool.tile([128, NG * 2, 2 * C], F16, tag=f"g{l}")
                seg = ch * NLVL + l
                # overlapping pair window: row stride C, window 2*C
                in_ap = bass.AP(tf_h, base * C, [[C, W * H - 1], [1, 2 * C]])
                nc.gpsimd.dma_gather(
                    out_ap=gt[:],
                    in_ap=in_ap,
                    idxs_ap=idxt[:, seg * 64 : (seg + 1) * 64],
                    num_idxs=NG * 256,
                    num_idxs_reg=NG * 256,
                    elem_size=2 * C,
                    elem_step=C,
                    queue_num=seg % 4,
                )
                gts.append(gt)
            lt = lts[ch]
            for pair in range(4):
                ps = ppool.tile([128, C], F32, tag="ps")
                for half in range(2):
                    g64 = pair * 2 + half
                    k = 0
                    for l in range(NLVL):
                        for s in range(2):
                            mm = nc.tensor.matmul(
                                ps[64 * half : 64 * half + 64, :],
                                lt[:, l * 16 + g64 * 2 + s, :],
                                gts[l][:, g64, s * C : (s + 1) * C],
                                start=(k == 0),
                                stop=(k == 2 * NLVL - 1),
                            )
                            # accumulation chains sharing a PSUM bank must
                            # not interleave -> force PE program order
                            if prev_mm is not None:
                                add_dep_helper(mm.ins, prev_mm.ins, sync=False)
                            prev_mm = mm
                            k += 1
                so = spool.tile([128, C], F16, tag="st")
                nc.scalar.activation(so[:], ps[:],
                                     mybir.ActivationFunctionType.Copy)
                row0 = ch * 512 + pair * 128
                nc.sync.dma_start(outd[row0 : row0 + 128, :], so[:])

    mybir.codegen_inst_isa_subclasses(nc)
    if fix:
        _fix_waits(nc)
    return nc


# ---------------------------------------------------------------------------
# Host-side prep

def _host_prep_points(center_b, boundary_b, roi0, nroi):
    """Returns (idx [128, NSEG*64] i16, wt [128, NCHUNK*64] f16) for one core."""
    bp = boundary_b[roi0 : roi0 + nroi]      # [nroi, Wp, 4]
    cp = center_b[roi0 : roi0 + nroi]        # [nroi, Wp, 2]
    sp = np.stack([bp[..., 0:2], cp, bp[..., 2:4]], axis=1)  # [nroi,3,Wp,2]
    gx = np.ascontiguousarray(sp[..., 0].transpose(1, 2, 0)).reshape(-1)
    gy = np.ascontiguousarray(sp[..., 1].transpose(1, 2, 0)).reshape(-1)
    gx = gx.astype(np.float32)
    gy = gy.astype(np.float32)

    q = np.arange(128)
    jj = q % 64
    tb = q // 64           # 0 = top pair (y0), 1 = bottom pair (y1)

    idx = np.zeros((128, NSEG * 64), np.int16)
    wt = np.zeros((128, NCHUNK * 64), np.float16)
    for l, (W, H, base) in enumerate(LEVELS):
        # match the reference's fp32 rounding: ((g + 1) * 0.5) * (W - 1)
        x = ((gx + np.float32(1.0)) * np.float32(0.5)) * np.float32(W - 1)
        y = ((gy + np.float32(1.0)) * np.float32(0.5)) * np.float32(H - 1)
        x0 = np.floor(x)
        y0 = np.floor(y)
        wx = x - x0
        wy = y - y0
        it = (y0 * W + x0).astype(np.int32)          # top-pair row index
        ib = it + W                                  # bottom-pair row index
        w00 = (1 - wx) * (1 - wy)                    # (x0, y0)
        w10 = wx * (1 - wy)                          # (x1, y0)
        w01 = (1 - wx) * wy                          # (x0, y1)
        w11 = wx * wy                                # (x1, y1)
        for ch in range(NCHUNK):
            seg = ch * NLVL + l
            # token t = g64*128 + tb*64 + j
            tok = np.empty((NG * 2, 128), np.int32)
            for g64 in range(8):
                p64 = ch * 512 + g64 * 64 + np.arange(64)
                tok[g64, :64] = it[p64]
                tok[g64, 64:] = ib[p64]
            flat = tok.reshape(-1)
            wrapped = flat.reshape(-1, 16).T.astype(np.int16)   # [16, 64]
            idx[:, seg * 64 : (seg + 1) * 64] = np.tile(wrapped, (8, 1))
            # weights: col l*16 + g64*2 + s, partition q = tb*64 + j
            for g64 in range(8):
                p128 = ch * 512 + g64 * 64 + jj
                ws0 = np.where(tb == 0, w00[p128], w01[p128])
                ws1 = np.where(tb == 0, w10[p128], w11[p128])
                wt[:, ch * 64 + l * 16 + g64 * 2 + 0] = ws0.astype(np.float16)
                wt[:, ch * 64 + l * 16 + g64 * 2 + 1] = ws1.astype(np.float16)
    return idx, wt


def _host_tfeats(feats_b_list):
    parts = [np.ascontiguousarray(f.reshape(f.shape[0], -1).T)
             for f in feats_b_list]
    tfx = np.concatenate(parts, axis=0)
    assert tfx.shape[0] == ROWS
    return np.ascontiguousarray(tfx.astype(np.float16))


def _host_mask():
    return (np.arange(128)[:, None] % 64
            == np.arange(64)[None, :]).astype(np.float16)


_CACHE = {}


def _get_nc():
    if "nc" not in _CACHE:
        _CACHE["nc"] = _build_kernel()
    return _CACHE["nc"]


def kernel(feats0, feats1, feats2, feats3, center_points, boundary_points,
           _want_trace=False, _trace_dir=None):
    feats0 = np.asarray(feats0, dtype=np.float32)
    feats1 = np.asarray(feats1, dtype=np.float32)
    feats2 = np.asarray(feats2, dtype=np.float32)
    feats3 = np.asarray(feats3, dtype=np.float32)
    center_points = np.asarray(center_points, dtype=np.float32)
    boundary_points = np.asarray(boundary_points, dtype=np.float32)

    nc = _get_nc()
    mask = _host_mask()
    tfeats = [
        _host_tfeats([feats0[b], feats1[b], feats2[b], feats3[b]])
        for b in range(BS)
    ]
    nroi = NROI_TOTAL // 4  # 64 rois per core
    in_maps = []
    for core in range(8):
        b = core // 4
        roi0 = (core % 4) * nroi
        idx, wt = _host_prep_points(
            center_points[b], boundary_points[b], roi0, nroi)
        in_maps.append(
            {"tfeats": tfeats[b], "idx": idx, "wt": wt, "mask": mask})

    kwargs = {}
    if _want_trace:
        kwargs = {"trace": True}
        if _trace_dir is not None:
            kwargs["tmpdir"] = _trace_dir
    res = run_bass_kernel_spmd(nc, in_maps, core_ids=list(range(8)), **kwargs)

    out = np.empty((BS, NROI_TOTAL, C, OUT_H, WP), np.float32)
    for core in range(8):
        b = core // 4
        roi0 = (core % 4) * nroi
        dev = res.results[core]["out"]          # [NPTS, C] f16, rows (h, w, roi')
        o = dev.astype(np.float32).reshape(OUT_H, WP, nroi, C)
        out[b, roi0 : roi0 + nroi] = o.transpose(2, 3, 0, 1)
    if _want_trace:
        return out, res
    return out


# revision 5
# speedup vs baseline: 2.1337x; 1.1142x over previous
"""Trainium (trn2) kernel for CurvedRoIExtractor (nn_CurvedRoIExtractor_28295244546862).

kernel(**inputs) takes the FULL inputs (as produced by setup_inputs()) and
returns the FULL output [2, 256, 256, 3, 16] f32.

Sharding: 8 cores = (batch b in {0,1}) x (64-roi quarter).  The core's
feature maps are pre-transposed on the host to a channel-last fp16 table
[34000, 256] (levels concatenated).  For every sample point the device
fetches the two ADJACENT-pixel pairs (x0,x1)@y0 and (x0,x1)@y1 per level
with nc.gpsimd.dma_gather — one 1 KB descriptor per pair (elem_size=512
fp16 elems, elem_step=256 so the pair windows overlap on the row grid).
Gather token order t = g64*128 + tb*64 + j puts a 64-point group's top
pairs in partitions 0-63 and bottom pairs in 64-127; the bilinear
weighted sum then runs on TensorE as matmuls with a two-band masked
lhsT[q, j] = (q%64==j) * w_{tb(q), side}[j] (host-precomputed weights,
lhsT built on DVE as mask x broadcast), accumulating the 8 (level, side)
slices per group into PSUM.  PSUM (f32) is staged to fp16 on the Scalar
engine into [128, 512] tiles (two 128-point blocks side by side -> 1 KB
DMA packets) and written out tile-major; the host reorders + upcasts.

The big prep tables (idx, weights) are themselves loaded with dma_gather
(row-replication for idx) because the hardware-DGE queue generates
128-partition DMAs slowly; only a 4 KB bootstrap idx table goes through
the slow path.  These two boot gathers double as ucode warmup.
"""

from contextlib import ExitStack

import numpy as np

import concourse.bass as bass
import concourse.mybir as mybir
import concourse.tile as tile
from concourse import library_config
from concourse.bass_utils import run_bass_kernel_spmd
from concourse.tile import add_dep_helper

F32 = mybir.dt.float32
F16 = mybir.dt.float16
I16 = mybir.dt.int16
AOP = mybir.AluOpType

# (W, H, base row) of each feature level inside the concatenated table
LEVELS = [
    (160, 160, 0),
    (80, 80, 25600),
    (40, 40, 32000),
    (20, 20, 33600),
]
ROWS = 34000
C = 256               # channels
BS = 2
NROI_TOTAL = 256
WP = 16
OUT_H = 3
NPTS = 3072           # per core: 64 rois * 3 * 16
NG = 4                # 128-token groups per gather (-> 1024-idx gathers)
NCHUNK = NPTS // (NG * 128)   # 6 chunks of 512 points
NLVL = len(LEVELS)
NSEG = NCHUNK * NLVL  # gather segments
ICOLS = NSEG * 64     # idx table cols
WCOLS = 64 + NCHUNK * 64  # mask + per-chunk weight cols (fp16), padded to 512
NOUT = NPTS // 256    # 12 output tiles of [128, 512]


def _fix_waits(nc, max_waits=1):
    """The walrus build in this env rejects >1 sem wait per instruction;
    spill extras onto preceding NOPs on the same engine."""
    for func in nc.m.functions:
        for bb in func.blocks:
            insts = bb.instructions
            for ins in list(insts):
                si = ins.sync_info
                if si is None:
                    continue
                w = list(si.on_wait)
                if len(w) > max_waits:
                    si.on_wait = w[:max_waits]
                    pos = insts.index(ins)
                    extra = w[max_waits:]
                    for k in range(0, len(extra), max_waits):
                        nop = mybir.InstNoOp(
                            name=f"{ins.name}-wf{k}",
                            engine=ins.engine,
                            bass_nofuse=True,
                            sync_info=mybir.SyncInfo(
                                on_wait=extra[k : k + max_waits], on_update=[]
                            ),
                        )
                        insts.insert(pos, nop)
                        pos += 1


def _build_kernel(fix=True):
    """Per-core program.  See module docstring for the layout."""
    nc = bass.Bass("TRN2", target_bir_lowering=False, num_devices=8,
                   num_swdge_queues=4)
    tf = nc.dram_tensor("tfeats", [ROWS, C], F16, kind="ExternalInput")
    bootd = nc.dram_tensor("boot", [128, 16], I16, kind="ExternalInput")
    idxd = nc.dram_tensor("idx", [16, ICOLS], I16, kind="ExternalInput")
    wmd = nc.dram_tensor("wm", [128, 512], F16, kind="ExternalInput")
    outd = nc.dram_tensor("out", [NOUT, 128, 2 * C], F16,
                          kind="ExternalOutput")
    tf_h = tf[:].tensor

    with tile.TileContext(nc) as tc, ExitStack() as ctx:
        prep = ctx.enter_context(tc.tile_pool(name="prep", bufs=1))
        gpool = ctx.enter_context(tc.tile_pool(name="g", bufs=3))
        opool = ctx.enter_context(tc.tile_pool(name="o", bufs=1))
        ppool = ctx.enter_context(tc.tile_pool(name="ps", bufs=6, space="PSUM"))

        nc.gpsimd.load_library(library_config.attnmlp)

        boot = prep.tile([128, 16], I16, tag="boot")
        idxt = prep.tile([128, 1, ICOLS], I16, tag="idx")
        wmt = prep.tile([128, 1, 512], F16, tag="wm")
        nc.sync.dma_start(boot[:], bootd[:])

        reg128 = nc.gpsimd.to_reg(128)
        reg1024 = nc.gpsimd.to_reg(NG * 256)

        # boot gather 1: replicate the wrapped idx table to 128 partitions
        nc.gpsimd.dma_gather(
            out_ap=idxt[:],
            in_ap=idxd[:],
            idxs_ap=boot[:, 0:8],
            num_idxs=128,
            num_idxs_reg=reg128,
            elem_size=ICOLS,
            queue_num=0,
        )
        # boot gather 2: per-partition mask+weights rows
        nc.gpsimd.dma_gather(
            out_ap=wmt[:],
            in_ap=wmd[:],
            idxs_ap=boot[:, 8:16],
            num_idxs=128,
            num_idxs_reg=reg128,
            elem_size=512,
            queue_num=1,
        )
        mask = wmt[:, 0, 0:64]

        # lhsT tiles for every chunk, built upfront on DVE:
        # lt[q, l*16 + g64*2 + s, j] = mask[q, j] * w[q, chunk-col]
        lts = []
        for ch in range(NCHUNK):
            lt = prep.tile([128, 4 * 16, 64], F16, tag=f"lt{ch}")
            nc.vector.tensor_tensor(
                lt[:],
                mask.unsqueeze(1).to_broadcast([128, 64, 64]),
                wmt[:, 0, 64 + ch * 64 : 128 + ch * 64].unsqueeze(2)
                    .to_broadcast([128, 64, 64]),
                AOP.mult,
            )
            lts.append(lt)

        prev_mm = None
        for ch in range(NCHUNK):
            gts = []
            for l, (W, H, base) in enumerate(LEVELS):
                gt = gpool.tile([128, NG * 2, 2 * C], F16, tag=f"g{l}")
                seg = ch * NLVL + l
                # overlapping pair window: row stride C, window 2*C
                in_ap = bass.AP(tf_h, base * C, [[C, W * H - 1], [1, 2 * C]])
                nc.gpsimd.dma_gather(
                    out_ap=gt[:],
                    in_ap=in_ap,
                    idxs_ap=idxt[:, 0, seg * 64 : (seg + 1) * 64],
                    num_idxs=NG * 256,
                    num_idxs_reg=reg1024,
                    elem_size=2 * C,
                    elem_step=C,
                    queue_num=seg % 4,
                )
                gts.append(gt)
            lt = lts[ch]
            for tpair in range(2):     # output tile = 2 point-pairs = 256 pts
                so = opool.tile([128, 2 * C], F16, tag=f"so{ch * 2 + tpair}")
                for h in range(2):
                    pair = tpair * 2 + h
                    ps = ppool.tile([128, C], F32, tag="ps")
                    for half in range(2):
                        g64 = pair * 2 + half
                        k = 0
                        for l in range(NLVL):
                            for s in range(2):
                                mm = nc.tensor.matmul(
                                    ps[64 * half : 64 * half + 64, :],
                                    lt[:, l * 16 + g64 * 2 + s, :],
                                    gts[l][:, g64, s * C : (s + 1) * C],
                                    start=(k == 0),
                                    stop=(k == 2 * NLVL - 1),
                                )
                                # accumulation chains sharing a PSUM bank
                                # must not interleave -> force PE order
                                if prev_mm is not None:
                                    add_dep_helper(mm.ins, prev_mm.ins,
                                                   sync=False)
                                prev_mm = mm
                                k += 1
                    nc.scalar.activation(so[:, h * C : (h + 1) * C], ps[:],
                                         mybir.ActivationFunctionType.Copy)
                nc.sync.dma_start(outd[ch * 2 + tpair], so[:])

    mybir.codegen_inst_isa_subclasses(nc)
    if fix:
        _fix_waits(nc)
    return nc


# ---------------------------------------------------------------------------
# Host-side prep

def _host_boot():
    """Bootstrap idx table [128, 16] i16: cols 0:8 feed the idx-replication
    gather (token t reads wrapped-idx row t%16), cols 8:16 the mask+weight
    gather (token t reads row t)."""
    r = np.arange(128)
    boot = np.empty((128, 16), np.int16)
    # wrapped [16, 8]: A[r, c] = token (c*16+r) % 16 = r ; B[r, c] = c*16+r
    A = np.tile(np.arange(16, dtype=np.int16)[:, None], (1, 8))
    B = (np.arange(8, dtype=np.int16)[None, :] * 16
         + np.arange(16, dtype=np.int16)[:, None])
    boot[:, 0:8] = np.tile(A, (8, 1))
    boot[:, 8:16] = np.tile(B, (8, 1))
    return boot


def _host_prep_points(center_b, boundary_b, roi0, nroi):
    """Returns (idx [16, ICOLS] i16, wm [128, 512] f16) for one core."""
    bp = boundary_b[roi0 : roi0 + nroi]      # [nroi, Wp, 4]
    cp = center_b[roi0 : roi0 + nroi]        # [nroi, Wp, 2]
    sp = np.stack([bp[..., 0:2], cp, bp[..., 2:4]], axis=1)  # [nroi,3,Wp,2]
    gx = np.ascontiguousarray(sp[..., 0].transpose(1, 2, 0)).reshape(-1)
    gy = np.ascontiguousarray(sp[..., 1].transpose(1, 2, 0)).reshape(-1)
    gx = gx.astype(np.float32)
    gy = gy.astype(np.float32)

    q = np.arange(128)
    jj = q % 64
    tb = q // 64           # 0 = top pair (y0), 1 = bottom pair (y1)

    idx = np.zeros((16, ICOLS), np.int16)
    wm = np.zeros((128, 512), np.float16)
    wm[:, 0:64] = (q[:, None] % 64 == np.arange(64)[None, :])
    for l, (W, H, base) in enumerate(LEVELS):
        # match the reference's fp32 rounding: ((g + 1) * 0.5) * (W - 1)
        x = ((gx + np.float32(1.0)) * np.float32(0.5)) * np.float32(W - 1)
        y = ((gy + np.float32(1.0)) * np.float32(0.5)) * np.float32(H - 1)
        x0 = np.floor(x)
        y0 = np.floor(y)
        wx = x - x0
        wy = y - y0
        it = (y0 * W + x0).astype(np.int32)          # top-pair row index
        ib = it + W                                  # bottom-pair row index
        w00 = (1 - wx) * (1 - wy)                    # (x0, y0)
        w10 = wx * (1 - wy)                          # (x1, y0)
        w01 = (1 - wx) * wy                          # (x0, y1)
        w11 = wx * wy                                # (x1, y1)
        for ch in range(NCHUNK):
            seg = ch * NLVL + l
            # token t = g64*128 + tb*64 + j
            tok = np.empty((NG * 2, 128), np.int32)
            for g64 in range(8):
                p64 = ch * 512 + g64 * 64 + np.arange(64)
                tok[g64, :64] = it[p64]
                tok[g64, 64:] = ib[p64]
            flat = tok.reshape(-1)
            idx[:, seg * 64 : (seg + 1) * 64] = \
                flat.reshape(-1, 16).T.astype(np.int16)
            # weights: col 64 + ch*64 + l*16 + g64*2 + s, partition q=tb*64+j
            for g64 in range(8):
                p128 = ch * 512 + g64 * 64 + jj
                ws0 = np.where(tb == 0, w00[p128], w01[p128])
                ws1 = np.where(tb == 0, w10[p128], w11[p128])
                col = 64 + ch * 64 + l * 16 + g64 * 2
                wm[:, col] = ws0.astype(np.float16)
                wm[:, col + 1] = ws1.astype(np.float16)
    return idx, wm


def _host_tfeats(feats_b_list):
    parts = [np.ascontiguousarray(f.reshape(f.shape[0], -1).T)
             for f in feats_b_list]
    tfx = np.concatenate(parts, axis=0)
    assert tfx.shape[0] == ROWS
    return np.ascontiguousarray(tfx.astype(np.float16))


_CACHE = {}


def _get_nc():
    if "nc" not in _CACHE:
        _CACHE["nc"] = _build_kernel()
    return _CACHE["nc"]


def kernel(feats0, feats1, feats2, feats3, center_points, boundary_points,
           _want_trace=False, _trace_dir=None):
    feats0 = np.asarray(feats0, dtype=np.float32)
    feats1 = np.asarray(feats1, dtype=np.float32)
    feats2 = np.asarray(feats2, dtype=np.float32)
    feats3 = np.asarray(feats3, dtype=np.float32)
    center_points = np.asarray(center_points, dtype=np.float32)
    boundary_points = np.asarray(boundary_points, dtype=np.float32)

    nc = _get_nc()
    boot = _host_boot()
    tfeats = [
        _host_tfeats([feats0[b], feats1[b], feats2[b], feats3[b]])
        for b in range(BS)
    ]
    nroi = NROI_TOTAL // 4  # 64 rois per core
    in_maps = []
    for core in range(8):
        b = core // 4
        roi0 = (core % 4) * nroi
        idx, wm = _host_prep_points(
            center_points[b], boundary_points[b], roi0, nroi)
        in_maps.append(
            {"tfeats": tfeats[b], "idx": idx, "wm": wm, "boot": boot})

    kwargs = {}
    if _want_trace:
        kwargs = {"trace": True}
        if _trace_dir is not None:
            kwargs["tmpdir"] = _trace_dir
    res = run_bass_kernel_spmd(nc, in_maps, core_ids=list(range(8)), **kwargs)

    out = np.empty((BS, NROI_TOTAL, C, OUT_H, WP), np.float32)
    for core in range(8):
        b = core // 4
        roi0 = (core % 4) * nroi
        dev = res.results[core]["out"]          # [12, 128, 512] f16
        pts = (dev.astype(np.float32)
               .reshape(NOUT, 128, 2, C)
               .transpose(0, 2, 1, 3)
               .reshape(NPTS, C))               # rows (h, w, roi')
        o = pts.reshape(OUT_H, WP, nroi, C)
        out[b, roi0 : roi0 + nroi] = o.transpose(2, 3, 0, 1)
    if _want_trace:
        return out, res
    return out


# revision 6
# speedup vs baseline: 2.3518x; 1.1022x over previous
"""Trainium (trn2) kernel for CurvedRoIExtractor (nn_CurvedRoIExtractor_28295244546862).

kernel(**inputs) takes the FULL inputs (as produced by setup_inputs()) and
returns the FULL output [2, 256, 256, 3, 16] f32.

Sharding: 8 cores = (batch b in {0,1}) x (64-roi quarter).  The core's
feature maps are pre-transposed on the host to a channel-last fp16 table
[34112, 256] (levels concatenated, zero-padded).  Levels 0-2: for every
sample point the device fetches the two ADJACENT-pixel pairs (x0,x1)@y0
and (x0,x1)@y1 with nc.gpsimd.dma_gather — one 1 KB descriptor per pair
(elem_size=512 fp16 elems, elem_step=256: overlapping pair windows).
Token order t = g64*128 + tb*64 + j puts a 64-point group's top pairs in
partitions 0-63, bottom pairs in 64-127; the bilinear weighted sum runs
on TensorE as matmuls with a two-band masked lhsT[q, j] = (q%64==j) *
w_{tb(q), side}[j] (lhsT built on DVE as mask x broadcast).  Level 3
(20x20) is tiny, so it is computed DENSELY instead of gathered: the
whole level-3 table (512 rows with pad) sits in SBUF and each 64-point
group adds 4 matmuls with a host-precomputed sparse-in-dense weight
matrix W3[pixel, point] — no per-point gather traffic at all.  All 10
matmuls per (group, chain) accumulate in PSUM; PSUM (f32) is staged to
fp16 on the Scalar engine into [128, 512] tiles (1 KB DMA packets) and
written out tile-major; the host reorders + upcasts.
"""

from contextlib import ExitStack

import numpy as np

import concourse.bass as bass
import concourse.mybir as mybir
import concourse.tile as tile
from concourse import library_config
from concourse.bass_utils import run_bass_kernel_spmd
from concourse.tile import add_dep_helper

F32 = mybir.dt.float32
F16 = mybir.dt.float16
I16 = mybir.dt.int16
AOP = mybir.AluOpType

# (W, H, base row) of each feature level inside the concatenated table
LEVELS = [
    (160, 160, 0),
    (80, 80, 25600),
    (40, 40, 32000),
    (20, 20, 33600),
]
NGLVL = 3             # levels gathered per point; level 3 handled densely
ROWS = 34112          # 34000 + 112 zero pad rows (level-3 tile pad)
C = 256               # channels
BS = 2
NROI_TOTAL = 256
WP = 16
OUT_H = 3
NPTS = 3072           # per core: 64 rois * 3 * 16
NG = 4                # 128-token groups per gather (-> 1024-idx gathers)
NCHUNK = NPTS // (NG * 128)   # 6 chunks of 512 points
NSEG = NCHUNK * NGLVL  # gather segments (18)
ICOLS = NSEG * 64 + 32 + 8  # chunk idx + t3 idx (512) + w3 idx (128)
NOUT = NPTS // 256    # 12 output tiles of [128, 512]
NG64 = NPTS // 64     # 48 64-point groups
W3COLS = NG64 * 4 * 64  # 12288


def _fix_waits(nc, max_waits=1):
    """The walrus build in this env rejects >1 sem wait per instruction;
    spill extras onto preceding NOPs on the same engine."""
    for func in nc.m.functions:
        for bb in func.blocks:
            insts = bb.instructions
            for ins in list(insts):
                si = ins.sync_info
                if si is None:
                    continue
                w = list(si.on_wait)
                if len(w) > max_waits:
                    si.on_wait = w[:max_waits]
                    pos = insts.index(ins)
                    extra = w[max_waits:]
                    for k in range(0, len(extra), max_waits):
                        nop = mybir.InstNoOp(
                            name=f"{ins.name}-wf{k}",
                            engine=ins.engine,
                            bass_nofuse=True,
                            sync_info=mybir.SyncInfo(
                                on_wait=extra[k : k + max_waits], on_update=[]
                            ),
                        )
                        insts.insert(pos, nop)
                        pos += 1


def _build_kernel(fix=True):
    """Per-core program.  See module docstring for the layout."""
    nc = bass.Bass("TRN2", target_bir_lowering=False, num_devices=8,
                   num_swdge_queues=4)
    tf = nc.dram_tensor("tfeats", [ROWS, C], F16, kind="ExternalInput")
    idxd = nc.dram_tensor("idx", [128, ICOLS], I16, kind="ExternalInput")
    wmd = nc.dram_tensor("wm", [128, 64 + NCHUNK * NGLVL * 16], F16,
                         kind="ExternalInput")
    w3d = nc.dram_tensor("w3", [128, W3COLS], F16, kind="ExternalInput")
    outd = nc.dram_tensor("out", [NOUT, 128, 2 * C], F16,
                          kind="ExternalOutput")
    tf_h = tf[:].tensor

    with tile.TileContext(nc) as tc, ExitStack() as ctx:
        prep = ctx.enter_context(tc.tile_pool(name="prep", bufs=1))
        gpool = ctx.enter_context(tc.tile_pool(name="g", bufs=3))
        opool = ctx.enter_context(tc.tile_pool(name="o", bufs=1))
        ppool = ctx.enter_context(tc.tile_pool(name="ps", bufs=6, space="PSUM"))

        nc.gpsimd.load_library(library_config.attnmlp)

        idxt = prep.tile([128, ICOLS], I16, tag="idx")
        wmt = prep.tile([128, 64 + NCHUNK * NGLVL * 16], F16, tag="wm")
        w3t = prep.tile([128, 1, W3COLS], F16, tag="w3")
        t3 = prep.tile([128, 4, C], F16, tag="t3")
        # chunk0-level0 idx first so its gather can launch ASAP
        nc.sync.dma_start(idxt[:, 0:64], idxd[:, 0:64])
        nc.sync.dma_start(idxt[:, 64:ICOLS], idxd[:, 64:ICOLS])
        nc.sync.dma_start(wmt[:], wmd[:])
        mask = wmt[:, 0:64]

        reg128 = nc.gpsimd.to_reg(128)
        reg512 = nc.gpsimd.to_reg(512)
        reg1024 = nc.gpsimd.to_reg(NG * 256)

        # lhsT tiles for every chunk, built upfront on DVE:
        # lt[q, l*16 + g64*2 + s, j] = mask[q, j] * w[q, chunk-col]
        lts = []
        for ch in range(NCHUNK):
            lt = prep.tile([128, NGLVL * 16, 64], F16, tag=f"lt{ch}")
            nc.vector.tensor_tensor(
                lt[:],
                mask.unsqueeze(1).to_broadcast([128, NGLVL * 16, 64]),
                wmt[:, 64 + ch * NGLVL * 16 : 64 + (ch + 1) * NGLVL * 16]
                    .unsqueeze(2).to_broadcast([128, NGLVL * 16, 64]),
                AOP.mult,
            )
            lts.append(lt)

        prev_mm = None
        first_loads_issued = False
        for ch in range(NCHUNK):
            gts = []
            for l in range(NGLVL):
                W, H, base = LEVELS[l]
                gt = gpool.tile([128, NG * 2, 2 * C], F16, tag=f"g{l}")
                seg = ch * NGLVL + l
                # overlapping pair window: row stride C, window 2*C
                in_ap = bass.AP(tf_h, base * C, [[C, W * H - 1], [1, 2 * C]])
                nc.gpsimd.dma_gather(
                    out_ap=gt[:],
                    in_ap=in_ap,
                    idxs_ap=idxt[:, seg * 64 : (seg + 1) * 64],
                    num_idxs=NG * 256,
                    num_idxs_reg=reg1024,
                    elem_size=2 * C,
                    elem_step=C,
                    queue_num=seg % 4,
                )
                gts.append(gt)
            if not first_loads_issued:
                # level-3 table + dense weights, loaded once via gathers
                # (the HWDGE queue generates 128-partition DMAs slowly)
                first_loads_issued = True
                b3 = LEVELS[3][2]
                nc.gpsimd.dma_gather(
                    out_ap=t3[:],
                    in_ap=bass.AP(tf_h, b3 * C, [[C, 512], [1, C]]),
                    idxs_ap=idxt[:, NSEG * 64 : NSEG * 64 + 32],
                    num_idxs=512,
                    num_idxs_reg=reg512,
                    elem_size=C,
                    queue_num=3,
                )
                nc.gpsimd.dma_gather(
                    out_ap=w3t[:],
                    in_ap=w3d[:],
                    idxs_ap=idxt[:, NSEG * 64 + 32 : NSEG * 64 + 40],
                    num_idxs=128,
                    num_idxs_reg=reg128,
                    elem_size=W3COLS,
                    queue_num=0,
                )
            lt = lts[ch]
            for tpair in range(2):     # output tile = 2 point-pairs = 256 pts
                so = opool.tile([128, 2 * C], F16, tag=f"so{ch * 2 + tpair}")
                for h in range(2):
                    pair = tpair * 2 + h
                    ps = ppool.tile([128, C], F32, tag="ps")
                    for half in range(2):
                        g64 = pair * 2 + half
                        g64g = ch * 8 + g64
                        k = 0
                        nmm = 2 * NGLVL + 4
                        for l in range(NGLVL):
                            for s in range(2):
                                mm = nc.tensor.matmul(
                                    ps[64 * half : 64 * half + 64, :],
                                    lt[:, l * 16 + g64 * 2 + s, :],
                                    gts[l][:, g64, s * C : (s + 1) * C],
                                    start=(k == 0),
                                    stop=(k == nmm - 1),
                                )
                                # accumulation chains sharing a PSUM bank
                                # must not interleave -> force PE order
                                if prev_mm is not None:
                                    add_dep_helper(mm.ins, prev_mm.ins,
                                                   sync=False)
                                prev_mm = mm
                                k += 1
                        for kt in range(4):   # dense level-3
                            off = (g64g * 4 + kt) * 64
                            mm = nc.tensor.matmul(
                                ps[64 * half : 64 * half + 64, :],
                                w3t[:, 0, off : off + 64],
                                t3[:, kt, :],
                                start=(k == 0),
                                stop=(k == nmm - 1),
                            )
                            add_dep_helper(mm.ins, prev_mm.ins, sync=False)
                            prev_mm = mm
                            k += 1
                    nc.scalar.activation(so[:, h * C : (h + 1) * C], ps[:],
                                         mybir.ActivationFunctionType.Copy)
                nc.sync.dma_start(outd[ch * 2 + tpair], so[:])

    mybir.codegen_inst_isa_subclasses(nc)
    if fix:
        _fix_waits(nc)
    return nc


# ---------------------------------------------------------------------------
# Host-side prep

def _wrap128(flat):
    """Token-order idx list -> wrapped [16, n/16] replicated to [128, ...]."""
    w = flat.reshape(-1, 16).T.astype(np.int16)
    return np.tile(w, (8, 1))


def _host_prep_points(center_b, boundary_b, roi0, nroi):
    """Returns (idx [128, ICOLS] i16, wm [128, .] f16, w3 [128, W3COLS] f16)."""
    bp = boundary_b[roi0 : roi0 + nroi]      # [nroi, Wp, 4]
    cp = center_b[roi0 : roi0 + nroi]        # [nroi, Wp, 2]
    sp = np.stack([bp[..., 0:2], cp, bp[..., 2:4]], axis=1)  # [nroi,3,Wp,2]
    gx = np.ascontiguousarray(sp[..., 0].transpose(1, 2, 0)).reshape(-1)
    gy = np.ascontiguousarray(sp[..., 1].transpose(1, 2, 0)).reshape(-1)
    gx = gx.astype(np.float32)
    gy = gy.astype(np.float32)

    q = np.arange(128)
    jj = q % 64
    tb = q // 64           # 0 = top pair (y0), 1 = bottom pair (y1)

    idx = np.zeros((128, ICOLS), np.int16)
    wm = np.zeros((128, 64 + NCHUNK * NGLVL * 16), np.float16)
    wm[:, 0:64] = (q[:, None] % 64 == np.arange(64)[None, :])

    def lvl_geom(l):
        W, H, base = LEVELS[l]
        x = ((gx + np.float32(1.0)) * np.float32(0.5)) * np.float32(W - 1)
        y = ((gy + np.float32(1.0)) * np.float32(0.5)) * np.float32(H - 1)
        x0 = np.floor(x)
        y0 = np.floor(y)
        wx = x - x0
        wy = y - y0
        return W, H, x0.astype(np.int32), y0.astype(np.int32), wx, wy

    for l in range(NGLVL):
        W, H, x0, y0, wx, wy = lvl_geom(l)
        it = y0 * W + x0
        ib = it + W
        w00 = (1 - wx) * (1 - wy)
        w10 = wx * (1 - wy)
        w01 = (1 - wx) * wy
        w11 = wx * wy
        for ch in range(NCHUNK):
            seg = ch * NGLVL + l
            tok = np.empty((NG * 2, 128), np.int32)
            for g64 in range(8):
                p64 = ch * 512 + g64 * 64 + np.arange(64)
                tok[g64, :64] = it[p64]
                tok[g64, 64:] = ib[p64]
            idx[:, seg * 64 : (seg + 1) * 64] = _wrap128(tok.reshape(-1))
            for g64 in range(8):
                p128 = ch * 512 + g64 * 64 + jj
                ws0 = np.where(tb == 0, w00[p128], w01[p128])
                ws1 = np.where(tb == 0, w10[p128], w11[p128])
                col = 64 + ch * NGLVL * 16 + l * 16 + g64 * 2
                wm[:, col] = ws0.astype(np.float16)
                wm[:, col + 1] = ws1.astype(np.float16)

    # t3 / w3 bootstrap idx
    idx[:, NSEG * 64 : NSEG * 64 + 32] = _wrap128(np.arange(512))
    idx[:, NSEG * 64 + 32 : NSEG * 64 + 40] = _wrap128(np.arange(128))

    # dense level-3 weights: W3[pix, pt] (512 pix rows with pad, 3072 pts)
    W, H, x0, y0, wx, wy = lvl_geom(3)
    w3full = np.zeros((512, NPTS), np.float32)
    pts = np.arange(NPTS)
    for dy, dx, wgt in ((0, 0, (1 - wx) * (1 - wy)), (0, 1, wx * (1 - wy)),
                        (1, 0, (1 - wx) * wy), (1, 1, wx * wy)):
        w3full[(y0 + dy) * W + (x0 + dx), pts] = wgt
    # w3[p, (g64*4 + k)*64 + j] = w3full[k*128 + p, g64*64 + j]
    w3 = np.ascontiguousarray(
        w3full.reshape(4, 128, NG64, 64)      # [k, p, g64, j]
        .transpose(1, 2, 0, 3)                # [p, g64, k, j]
        .reshape(128, W3COLS)).astype(np.float16)
    return idx, wm, w3


def _host_tfeats(feats_b_list):
    parts = [np.ascontiguousarray(f.reshape(f.shape[0], -1).T)
             for f in feats_b_list]
    tfx = np.concatenate(parts, axis=0)
    pad = ROWS - tfx.shape[0]
    tfx = np.concatenate([tfx, np.zeros((pad, C), tfx.dtype)], axis=0)
    return np.ascontiguousarray(tfx.astype(np.float16))


_CACHE = {}


def _get_nc():
    if "nc" not in _CACHE:
        _CACHE["nc"] = _build_kernel()
    return _CACHE["nc"]


def kernel(feats0, feats1, feats2, feats3, center_points, boundary_points,
           _want_trace=False, _trace_dir=None):
    feats0 = np.asarray(feats0, dtype=np.float32)
    feats1 = np.asarray(feats1, dtype=np.float32)
    feats2 = np.asarray(feats2, dtype=np.float32)
    feats3 = np.asarray(feats3, dtype=np.float32)
    center_points = np.asarray(center_points, dtype=np.float32)
    boundary_points = np.asarray(boundary_points, dtype=np.float32)

    nc = _get_nc()
    tfeats = [
        _host_tfeats([feats0[b], feats1[b], feats2[b], feats3[b]])
        for b in range(BS)
    ]
    nroi = NROI_TOTAL // 4  # 64 rois per core
    in_maps = []
    for core in range(8):
        b = core // 4
        roi0 = (core % 4) * nroi
        idx, wm, w3 = _host_prep_points(
            center_points[b], boundary_points[b], roi0, nroi)
        in_maps.append(
            {"tfeats": tfeats[b], "idx": idx, "wm": wm, "w3": w3})

    kwargs = {}
    if _want_trace:
        kwargs = {"trace": True}
        if _trace_dir is not None:
            kwargs["tmpdir"] = _trace_dir
    res = run_bass_kernel_spmd(nc, in_maps, core_ids=list(range(8)), **kwargs)

    out = np.empty((BS, NROI_TOTAL, C, OUT_H, WP), np.float32)
    for core in range(8):
        b = core // 4
        roi0 = (core % 4) * nroi
        dev = res.results[core]["out"]          # [12, 128, 512] f16
        pts = (dev.astype(np.float32)
               .reshape(NOUT, 128, 2, C)
               .transpose(0, 2, 1, 3)
               .reshape(NPTS, C))               # rows (h, w, roi')
        o = pts.reshape(OUT_H, WP, nroi, C)
        out[b, roi0 : roi0 + nroi] = o.transpose(2, 3, 0, 1)
    if _want_trace:
        return out, res
    return out
